# revision 22
# baseline (speedup 1.0000x reference)
"""Trainium2 Bass kernel for nn_DownBlock (binary conv downblock).

Reference semantics (forward values only):
  b1   = silu(emb) @ m1_w.T + m1_b                      # [B, Cin]
  act  = sign(x + b1[:, :, None, None])                 # in {-1, 0, +1}
  xp   = avgpool2x2(x)                                  # [B, Cin, 32, 32]
  for i in 0, 1:
      R_i = conv3x3_s2(act, sign(conv_w[i]))            # integer-valued
      # y = sf*R + conv_b with sf = mean|conv_w[i]| per out-channel
      # BN(y) = A*R + (beta - A*mean_R)  with
      #   A = sf*gamma*rsqrt(sf^2*var_R + eps)          (conv_b cancels)
      o_i = A_i*R_i + D_i + xp
  o = concat(o_0, o_1) + b2 ; o = prelu(o) ; o = o + b3

Sharding: data-parallel over batch, 4 images per core, weights replicated.
BN batch stats (sum, sumsq of R per channel) are all-reduced across the 8
cores (2 x [128,2] fp32 collectives, one per conv half).

The conv runs as 9 accumulating bf16 matmuls per output PSUM tile over
parity-split, zero-padded activation planes; +-1 operands make the matmul
arithmetic exact (integer accumulation in fp32 PSUM).
"""

import sys
from contextlib import ExitStack

import numpy as np

if "/opt/trn_rl_repo" not in sys.path:
    sys.path.insert(0, "/opt/trn_rl_repo")

import concourse.bass as bass  # noqa: E402
import concourse.bacc as bacc  # noqa: E402
import concourse.tile as tile  # noqa: E402
from concourse import mybir  # noqa: E402

F32 = mybir.dt.float32
F16 = mybir.dt.float16
F8 = mybir.dt.float8e4
BF16 = mybir.dt.bfloat16
AF = mybir.ActivationFunctionType
ALU = mybir.AluOpType
AX = mybir.AxisListType

# Problem shape (hardcoded per contract)
B, CIN, H, W = 32, 128, 64, 64
E, P, SC = 512, 256, 2
NC_CORES = 8
BPC = B // NC_CORES            # 4 images per core
HO, WO = H // 2, W // 2        # 32, 32
NTOT = B * HO * WO             # 32768 global positions per channel
BN_EPS = 1e-5
TILE_N = 512                   # PSUM tile free size
TPI = HO * WO // TILE_N        # PSUM tiles per image per half = 2
CH = HO * WO                   # 1024 positions per image
USE_CC = True                  # all-reduce vs local-copy (debug)


def build_kernel_body(tc, stage=3):
    nc = tc.nc
    x_d = nc.dram_tensor("x", [BPC, CIN, H, W], F32, kind="ExternalInput")
    embT_d = nc.dram_tensor("embT", [E, BPC], F32, kind="ExternalInput")
    wT_d = nc.dram_tensor("conv_wT", [SC, 3, 3, CIN, CIN], F32,
                          kind="ExternalInput")       # [i, kh, kw, ci, co]
    mT_d = nc.dram_tensor("mT", [E, 640], F32, kind="ExternalInput")
    consts_d = nc.dram_tensor("consts", [CIN, 11], F32, kind="ExternalInput")
    out_d = nc.dram_tensor("out", [BPC, P, HO, WO], F32, kind="ExternalOutput")

    with ExitStack() as ctx:
        singles = ctx.enter_context(tc.tile_pool(name="singles", bufs=1))
        xpool = ctx.enter_context(tc.tile_pool(name="xpool", bufs=4))
        t1pool = ctx.enter_context(tc.tile_pool(name="t1pool", bufs=3))
        pspool = ctx.enter_context(tc.tile_pool(name="pspool", bufs=6, space="PSUM"))
        psmall = ctx.enter_context(tc.tile_pool(name="psmall", bufs=2, space="PSUM"))
        ttrp = ctx.enter_context(tc.tile_pool(name="ttrp", bufs=2))
        normp = ctx.enter_context(tc.tile_pool(name="normp", bufs=2))
        drampool = ctx.enter_context(tc.tile_pool(name="drampool", bufs=1, space="DRAM"))

        # Dummy collective first: absorbs CC-core cold-start while the
        # preamble runs, so the real stats all-reduce is cheaper.
        if USE_CC:
            dumm_i = drampool.tile([1, 4], F32, tag="dumm_i", name="dumm_i")
            dumm_o = drampool.tile([1, 4], F32, tag="dumm_o", name="dumm_o")
            zt = singles.tile([1, 4], F32)
            nc.gpsimd.memset(zt, 0.0)
            nc.gpsimd.dma_start(out=dumm_i[:, :], in_=zt)
            nc.gpsimd.collective_compute(
                "AllReduce", ALU.add,
                replica_groups=[list(range(NC_CORES))],
                ins=[dumm_i[:, :].opt()], outs=[dumm_o[:, :].opt()],
            )

        # ---------------- preamble: consts, emb, bias matmuls ----------------
        consts = singles.tile([CIN, 11], F32)
        nc.sync.dma_start(out=consts, in_=consts_d[:, :])

        embT = singles.tile([128, 4, BPC], F32)   # [E-chunk part, chunk, n]
        nc.sync.dma_start(out=embT,
                          in_=embT_d.rearrange("(c p) n -> p c n", p=128))
        sg = singles.tile([128, 4, BPC], F32)
        nc.scalar.activation(sg, embT, AF.Sigmoid)
        semb = singles.tile([128, 4, BPC], F32)   # silu(emb)^T
        nc.vector.tensor_mul(semb, embT, sg)

        mT = singles.tile([128, 4, 640], F32)
        nc.sync.dma_start(out=mT, in_=mT_d.rearrange("(c p) m -> p c m", p=128))

        # bias_all[c, blk, n]: blk 0 -> b1; 1,2 -> b2 halves; 3,4 -> b3 halves
        bias_all = singles.tile([CIN, 5, BPC], F32)
        for blk in range(5):
            pb = psmall.tile([CIN, BPC], F32, tag="pp", name="pb")
            for e in range(4):
                nc.tensor.matmul(
                    pb, lhsT=mT[:, e, blk * 128:(blk + 1) * 128],
                    rhs=semb[:, e, :], start=(e == 0), stop=(e == 3),
                )
            nc.vector.tensor_copy(out=bias_all[:, blk, :], in_=pb)
        b1 = singles.tile([CIN, BPC], F32)
        nc.vector.tensor_scalar(b1, bias_all[:, 0, :], consts[:, 0:1], None, ALU.add)
        b2 = singles.tile([CIN, SC, BPC], F32)
        b3 = singles.tile([CIN, SC, BPC], F32)
        for i in range(SC):
            nc.vector.tensor_scalar(
                b2[:, i, :], bias_all[:, 1 + i, :], consts[:, 1 + i:2 + i], None, ALU.add)
            nc.vector.tensor_scalar(
                b3[:, i, :], bias_all[:, 3 + i, :], consts[:, 3 + i:4 + i], None, ALU.add)

        # ---------------- conv weights: load, sign, |w| means ----------------
        wraw = singles.tile([CIN, SC, 3, 3, CIN], F32)   # [ci, i, kh, kw, co]
        nc.sync.dma_start(
            out=wraw,
            in_=wT_d[:, :, :, :, :].rearrange("i kh kw ci co -> ci i kh kw co"))
        # DoubleRow weights: rows kh=0,2 paired per kw; kh=1 taps solo
        wdr = singles.tile([CIN, SC, 3, 2, CIN], F8)
        wsolo = singles.tile([CIN, SC, 3, CIN], F8)
        for i in range(SC):
            for kw in range(3):
                nc.scalar.activation(wdr[:, i, kw, 0, :], wraw[:, i, 0, kw, :], AF.Sign)
                nc.scalar.activation(wdr[:, i, kw, 1, :], wraw[:, i, 2, kw, :], AF.Sign)
                nc.scalar.activation(wsolo[:, i, kw, :], wraw[:, i, 1, kw, :], AF.Sign)
        wabs = singles.tile([CIN, SC, 3, 3, CIN], BF16)  # |w|
        nc.scalar.activation(wabs, wraw, AF.Abs)
        ones_vec = singles.tile([CIN, 1], BF16)
        nc.vector.memset(ones_vec, 1.0)
        sf = singles.tile([CIN, SC], F32)                # per-co mean|w|
        for i in range(SC):
            sfp = psmall.tile([CIN, BPC], F32, tag="pp", name="sfp")[:, 0:1]
            for t in range(9):
                kh, kw = t // 3, t % 3
                nc.tensor.matmul(
                    sfp, lhsT=wabs[:, i, kh, kw, :], rhs=ones_vec,
                    start=(t == 0), stop=(t == 8),
                )
            nc.scalar.activation(sf[:, i:i + 1], sfp, AF.Copy, scale=1.0 / 1152.0)

        # ---------------- activation planes + pooling ----------------
        # parity planes of act = sign(x + b1); odd planes zero-padded at idx 0
        pl_ee = singles.tile([CIN, BPC, 32, 32], F8)
        pl_eo = singles.tile([CIN, BPC, 32, 33], F8)
        pl_oe = singles.tile([CIN, BPC, 33, 32], F8)
        pl_oo = singles.tile([CIN, BPC, 33, 33], F8)
        for n in range(BPC):
            nc.gpsimd.memset(pl_eo[:, n, :, 0:1], 0.0)
            nc.gpsimd.memset(pl_oe[:, n, 0:1, :], 0.0)
            nc.gpsimd.memset(pl_oo[:, n, 0:1, :], 0.0)
            nc.gpsimd.memset(pl_oo[:, n, 1:33, 0:1], 0.0)

        xpsum = singles.tile([CIN, BPC, HO, WO], F32)    # 4 * avgpool2(x)

        # tap (kh, kw) -> plane and index offsets (see module docstring)
        def tap_ap(kh, kw, n, r0):
            rodd = kh != 1
            codd = kw != 1
            dr = 1 if kh == 2 else 0
            dc = 1 if kw == 2 else 0
            pl = {(False, False): pl_ee, (False, True): pl_eo,
                  (True, False): pl_oe, (True, True): pl_oo}[(rodd, codd)]
            nrow = TILE_N // WO  # 16
            return pl[:, n, r0 + dr:r0 + dr + nrow, dc:dc + WO]

        R = [singles.tile([CIN, BPC, HO * WO], F16, tag=f"R{i}", name=f"R{i}") for i in range(SC)]
        Sp = [singles.tile([CIN, BPC * TPI], F32, tag=f"Sp{i}", name=f"Sp{i}") for i in range(SC)]
        Qp = [singles.tile([CIN, BPC * TPI], F32, tag=f"Qp{i}", name=f"Qp{i}") for i in range(SC)]
        gstat4 = singles.tile([CIN, 4], F32)      # [S0,Q0,S1,Q1] global
        loc4 = singles.tile([CIN, 4], F32)
        cc_in4 = drampool.tile([CIN, 4], F32, tag="cci", name="cci")
        cc_out4 = drampool.tile([CIN, 4], F32, tag="cco", name="cco")

        NR = TILE_N // WO  # 16 output rows per PSUM tile

        def conv_tile(i, n, hb):
            ps = pspool.tile([CIN, TILE_N], F32, tag="ps", name="ps")
            r0 = hb * NR
            for kw in range(3):
                # rows kh=0 (idx r0) and kh=2 (idx r0+1) as a DoubleRow pair
                codd = kw != 1
                dc = 1 if kw == 2 else 0
                pl = pl_oo if codd else pl_oe
                base = pl[:, n, r0:r0 + NR, dc:dc + WO]
                rs = pl.shape[3]  # plane row stride in elements
                pair = bass.AP(
                    tensor=base.tensor, offset=base.offset,
                    ap=[list(base.ap[0]), [rs, 2]] + [list(x) for x in base.ap[1:]])
                nc.tensor.matmul(
                    ps, lhsT=wdr[:, i, kw, :, :], rhs=pair,
                    perf_mode=mybir.MatmulPerfMode.DoubleRow,
                    start=(kw == 0), stop=False,
                )
            for kw in range(3):
                nc.tensor.matmul(
                    ps, lhsT=wsolo[:, i, kw, :],
                    rhs=tap_ap(1, kw, n, r0),
                    start=False, stop=(kw == 2),
                )
            ti = n * TPI + hb
            rslice = R[i][:, n, hb * TILE_N:(hb + 1) * TILE_N]
            nc.scalar.activation(rslice, ps, AF.Identity,
                                 accum_out=Sp[i][:, ti:ti + 1])
            scr = ttrp.tile([CIN, TILE_N], BF16, tag="ttr", name="scr")
            nc.scalar.activation(scr, ps, AF.Square,
                                 accum_out=Qp[i][:, ti:ti + 1])

        # i=0 conv fused with x streaming: per image load x, make planes
        # (ACT) + pool (DVE), then conv both PSUM tiles. Keeps the ACT
        # queue free of head-of-line blocking on later x DMAs.
        for n in range(BPC):
            x_t = xpool.tile([CIN, H, W], F32, tag="x_t", name=f"x_t{n}")
            nc.sync.dma_start(out=x_t, in_=x_d[n, :, :, :])
            xr = x_t.rearrange("p (h a) (w b) -> p h a w b", a=2, b=2)
            bn_ap = b1[:, n:n + 1]
            nc.scalar.activation(pl_ee[:, n, :, :], xr[:, :, 0, :, 0],
                                 AF.Sign, bias=bn_ap)
            nc.scalar.activation(pl_eo[:, n, :, 1:33], xr[:, :, 0, :, 1],
                                 AF.Sign, bias=bn_ap)
            nc.scalar.activation(pl_oe[:, n, 1:33, :], xr[:, :, 1, :, 0],
                                 AF.Sign, bias=bn_ap)
            nc.scalar.activation(pl_oo[:, n, 1:33, 1:33], xr[:, :, 1, :, 1],
                                 AF.Sign, bias=bn_ap)
            # pooling on DVE (idle during the conv phase)
            xw = x_t.rearrange("p h (w b) -> p h w b", b=2)
            t1 = t1pool.tile([CIN, H, WO], F32, tag="t1", name="t1")
            nc.vector.tensor_add(t1, xw[:, :, :, 0], xw[:, :, :, 1])
            t1r = t1.rearrange("p (h a) w -> p h a w", a=2)
            nc.vector.tensor_add(xpsum[:, n, :, :], t1r[:, :, 0, :], t1r[:, :, 1, :])
            for i in range(SC):
                for hb in range(TPI):
                    conv_tile(i, n, hb)

        for i in range(SC):
            nc.vector.reduce_sum(out=loc4[:, 2 * i:2 * i + 1], in_=Sp[i], axis=AX.X)
            nc.vector.reduce_sum(out=loc4[:, 2 * i + 1:2 * i + 2], in_=Qp[i], axis=AX.X)

        nc.gpsimd.dma_start(out=cc_in4[:, :], in_=loc4)
        if USE_CC:
            nc.gpsimd.collective_compute(
                "AllReduce", ALU.add,
                replica_groups=[list(range(NC_CORES))],
                ins=[cc_in4[:, :].opt()], outs=[cc_out4[:, :].opt()],
            )
            nc.gpsimd.dma_start(out=gstat4, in_=cc_out4[:, :])
        else:
            nc.gpsimd.dma_start(out=gstat4, in_=cc_in4[:, :])

        if stage < 3:
            # dump R halves to out and stop
            for i in range(SC):
                for n in range(BPC):
                    dump = normp.tile([CIN, CH], F32, tag="xpD")
                    nc.vector.tensor_copy(out=dump, in_=R[i][:, n, :])
                    nc.sync.dma_start(
                        out=out_d[n, i * CIN:(i + 1) * CIN, :, :].rearrange(
                            "c h w -> c (h w)"),
                        in_=dump)
            return
        # ---------------- finalize per-channel A, D ----------------
        A = [singles.tile([CIN, 1], F32, tag=f"A{i}", name=f"A{i}") for i in range(SC)]
        Dn = [singles.tile([CIN, BPC], F32, tag=f"Dn{i}", name=f"Dn{i}") for i in range(SC)]
        for i in range(SC):
            tmp = singles.tile([CIN, 8], F32, tag=f"tmp{i}")
            mean = tmp[:, 6:7]
            e2 = tmp[:, 7:8]
            nc.scalar.activation(mean, gstat4[:, 2 * i:2 * i + 1], AF.Copy, scale=1.0 / NTOT)
            nc.scalar.activation(e2, gstat4[:, 2 * i + 1:2 * i + 2], AF.Copy, scale=1.0 / NTOT)
            # var = e2 - mean^2
            nc.vector.tensor_mul(tmp[:, 0:1], mean, mean)
            nc.vector.tensor_sub(tmp[:, 0:1], e2, tmp[:, 0:1])
            # z = var * sf^2 + eps
            nc.vector.tensor_mul(tmp[:, 1:2], sf[:, i:i + 1], sf[:, i:i + 1])
            nc.vector.tensor_mul(tmp[:, 1:2], tmp[:, 0:1], tmp[:, 1:2])
            nc.vector.tensor_scalar(tmp[:, 1:2], tmp[:, 1:2], BN_EPS, None, ALU.add)
            # r = 1/sqrt(z) via sqrt+reciprocal, then one Newton step:
            # r <- r * (1.5 - 0.5 * z * r^2)
            nc.scalar.activation(tmp[:, 2:3], tmp[:, 1:2], AF.Sqrt)
            nc.vector.reciprocal(tmp[:, 2:3], tmp[:, 2:3])
            nc.vector.tensor_mul(tmp[:, 3:4], tmp[:, 2:3], tmp[:, 2:3])
            nc.vector.tensor_mul(tmp[:, 3:4], tmp[:, 3:4], tmp[:, 1:2])
            nc.vector.tensor_scalar(tmp[:, 3:4], tmp[:, 3:4], -0.5, 1.5,
                                    ALU.mult, ALU.add)
            nc.vector.tensor_mul(tmp[:, 2:3], tmp[:, 2:3], tmp[:, 3:4])
            # A = gamma * sf * rstd
            nc.vector.tensor_mul(tmp[:, 4:5], consts[:, 5 + i:6 + i], sf[:, i:i + 1])
            nc.vector.tensor_mul(A[i], tmp[:, 4:5], tmp[:, 2:3])
            # Dn = bn_b - A*mean + b2      (per n)
            nc.vector.tensor_mul(tmp[:, 5:6], A[i], mean)
            nc.vector.scalar_tensor_tensor(
                out=tmp[:, 5:6], in0=tmp[:, 5:6], scalar=-1.0,
                in1=consts[:, 7 + i:8 + i], op0=ALU.mult, op1=ALU.add)
            nc.vector.tensor_scalar(Dn[i], b2[:, i, :], tmp[:, 5:6], None, ALU.add)

        # ---------------- normalize + prelu + bias3 + store ----------------
        for i in range(SC):
            a_ap = consts[:, 9 + i:10 + i]
            for n in range(BPC):
                k = i * BPC + n
                # fp16 intermediates: exact R, ~1e-3 abs noise on the
                # residual path - far below the binary-activation noise
                # floor; buys DVE 2x mode on the two STT passes.
                xpD = normp.tile([CIN, CH], F16, tag="xpD")
                xps = xpsum[:, n, :, :].rearrange("c h w -> c (h w)")
                nc.scalar.activation(xpD, xps, AF.Identity,
                                     bias=Dn[i][:, n:n + 1], scale=0.25)
                v = normp.tile([CIN, CH], F16, tag="v")
                nc.vector.scalar_tensor_tensor(
                    out=v, in0=R[i][:, n, :], scalar=A[i], in1=xpD,
                    op0=ALU.mult, op1=ALU.add)
                u = normp.tile([CIN, CH], F16, tag="u")
                nc.vector.scalar_tensor_tensor(
                    out=u, in0=v, scalar=a_ap, in1=v, op0=ALU.mult, op1=ALU.max)
                o = normp.tile([CIN, CH], F32, tag="o")
                if k % 2 == 0:
                    nc.vector.tensor_scalar(o, u, b3[:, i, n:n + 1], None, ALU.add)
                else:
                    nc.scalar.activation(o, u, AF.Identity, bias=b3[:, i, n:n + 1])
                nc.sync.dma_start(
                    out=out_d[n, i * CIN:(i + 1) * CIN, :, :].rearrange(
                        "c h w -> c (h w)"),
                    in_=o)


def build_nc(stage=3):
    nc = bacc.Bacc("TRN2", target_bir_lowering=False, debug=False)
    with tile.TileContext(nc) as tc:
        build_kernel_body(tc, stage)
    nc.finalize()
    return nc


def _prep_in_maps(x, emb, m1_w, m1_b, conv_w, conv_b, bn_g, bn_b,
                  m2_w, m2_b, prelu_a, m3_w, m3_b):
    """Host-side sharding + pure relayout (no arithmetic on tensor data)."""
    x = np.asarray(x, np.float32)
    conv_wT = np.ascontiguousarray(
        np.transpose(np.asarray(conv_w, np.float32), (0, 3, 4, 2, 1)))
    mT = np.ascontiguousarray(np.concatenate(
        [np.asarray(m1_w), np.asarray(m2_w), np.asarray(m3_w)], axis=0,
        dtype=np.float32).T)
    consts = np.zeros((CIN, 11), np.float32)
    consts[:, 0] = m1_b
    consts[:, 1] = m2_b[:CIN]
    consts[:, 2] = m2_b[CIN:]
    consts[:, 3] = m3_b[:CIN]
    consts[:, 4] = m3_b[CIN:]
    consts[:, 5] = bn_g[0]
    consts[:, 6] = bn_g[1]
    consts[:, 7] = bn_b[0]
    consts[:, 8] = bn_b[1]
    consts[:, 9] = prelu_a[:CIN]
    consts[:, 10] = prelu_a[CIN:]
    embT = np.ascontiguousarray(np.asarray(emb, np.float32).T)   # [E, B]
    in_maps = []
    for c in range(NC_CORES):
        in_maps.append({
            "x": np.ascontiguousarray(x[c * BPC:(c + 1) * BPC]),
            "embT": np.ascontiguousarray(embT[:, c * BPC:(c + 1) * BPC]),
            "conv_wT": conv_wT,
            "mT": mT,
            "consts": consts,
        })
    return in_maps


_CACHE = {}


def kernel(**inputs):
    from concourse.bass_utils import run_bass_kernel_spmd
    if "nc" not in _CACHE:
        _CACHE["nc"] = build_nc()
    nc = _CACHE["nc"]
    in_maps = _prep_in_maps(**{k: np.asarray(v) for k, v in inputs.items()})
    res = run_bass_kernel_spmd(nc, in_maps, list(range(NC_CORES)))
    return np.concatenate([res.results[c]["out"] for c in range(NC_CORES)], axis=0)


if __name__ == "__main__":
    build_nc()
    print("kernel build ok")


# revision 24
# speedup vs baseline: 1.1149x; 1.1149x over previous
"""Trainium2 Bass kernel for nn_DownBlock (binary conv downblock).

Reference semantics (forward values only):
  b1   = silu(emb) @ m1_w.T + m1_b                      # [B, Cin]
  act  = sign(x + b1[:, :, None, None])                 # in {-1, 0, +1}
  xp   = avgpool2x2(x)                                  # [B, Cin, 32, 32]
  for i in 0, 1:
      R_i = conv3x3_s2(act, sign(conv_w[i]))            # integer-valued
      # y = sf*R + conv_b with sf = mean|conv_w[i]| per out-channel
      # BN(y) = A*R + (beta - A*mean_R)  with
      #   A = sf*gamma*rsqrt(sf^2*var_R + eps)          (conv_b cancels)
      o_i = A_i*R_i + D_i + xp
  o = concat(o_0, o_1) + b2 ; o = prelu(o) ; o = o + b3

Sharding: data-parallel over batch, 4 images per core, weights replicated.
BN batch stats (sum, sumsq of R per channel) are all-reduced across the 8
cores (2 x [128,2] fp32 collectives, one per conv half).

The conv runs as 9 accumulating bf16 matmuls per output PSUM tile over
parity-split, zero-padded activation planes; +-1 operands make the matmul
arithmetic exact (integer accumulation in fp32 PSUM).
"""

import sys
from contextlib import ExitStack

import numpy as np

if "/opt/trn_rl_repo" not in sys.path:
    sys.path.insert(0, "/opt/trn_rl_repo")

import concourse.bass as bass  # noqa: E402
import concourse.bacc as bacc  # noqa: E402
import concourse.tile as tile  # noqa: E402
from concourse import mybir  # noqa: E402

F32 = mybir.dt.float32
F16 = mybir.dt.float16
F8 = mybir.dt.float8e4
BF16 = mybir.dt.bfloat16
AF = mybir.ActivationFunctionType
ALU = mybir.AluOpType
AX = mybir.AxisListType

# Problem shape (hardcoded per contract)
B, CIN, H, W = 32, 128, 64, 64
E, P, SC = 512, 256, 2
NC_CORES = 8
BPC = B // NC_CORES            # 4 images per core
HO, WO = H // 2, W // 2        # 32, 32
NTOT = B * HO * WO             # 32768 global positions per channel
BN_EPS = 1e-5
TILE_N = 512                   # PSUM tile free size
TPI = HO * WO // TILE_N        # PSUM tiles per image per half = 2
CH = HO * WO                   # 1024 positions per image
USE_CC = True                  # all-reduce vs local-copy (debug)


def build_kernel_body(tc, stage=3):
    nc = tc.nc
    x_d = nc.dram_tensor("x", [BPC, CIN, H, W], F32, kind="ExternalInput")
    embT_d = nc.dram_tensor("embT", [E, BPC], F32, kind="ExternalInput")
    wT_d = nc.dram_tensor("conv_wT", [SC, 3, 3, CIN, CIN], F32,
                          kind="ExternalInput")       # [i, kh, kw, ci, co]
    mT_d = nc.dram_tensor("mT", [E, 640], F32, kind="ExternalInput")
    consts_d = nc.dram_tensor("consts", [CIN, 11], F32, kind="ExternalInput")
    out_d = nc.dram_tensor("out", [BPC, P, HO, WO], F32, kind="ExternalOutput")

    with ExitStack() as ctx:
        singles = ctx.enter_context(tc.tile_pool(name="singles", bufs=1))
        xpool = ctx.enter_context(tc.tile_pool(name="xpool", bufs=4))
        t1pool = ctx.enter_context(tc.tile_pool(name="t1pool", bufs=3))
        pspool = ctx.enter_context(tc.tile_pool(name="pspool", bufs=6, space="PSUM"))
        psmall = ctx.enter_context(tc.tile_pool(name="psmall", bufs=2, space="PSUM"))
        ttrp = ctx.enter_context(tc.tile_pool(name="ttrp", bufs=2))
        normp = ctx.enter_context(tc.tile_pool(name="normp", bufs=2))
        drampool = ctx.enter_context(tc.tile_pool(name="drampool", bufs=1, space="DRAM"))

        # Dummy collective first: absorbs CC-core cold-start while the
        # preamble runs, so the real stats all-reduce is cheaper.
        if USE_CC:
            dumm_i = drampool.tile([1, 4], F32, tag="dumm_i", name="dumm_i")
            dumm_o = drampool.tile([1, 4], F32, tag="dumm_o", name="dumm_o")
            zt = singles.tile([1, 4], F32)
            nc.gpsimd.memset(zt, 0.0)
            nc.gpsimd.dma_start(out=dumm_i[:, :], in_=zt)
            nc.gpsimd.collective_compute(
                "AllReduce", ALU.add,
                replica_groups=[list(range(NC_CORES))],
                ins=[dumm_i[:, :].opt()], outs=[dumm_o[:, :].opt()],
            )

        # ---------------- preamble: consts, emb, bias matmuls ----------------
        consts = singles.tile([CIN, 11], F32)
        nc.sync.dma_start(out=consts, in_=consts_d[:, :])

        embT = singles.tile([128, 4, BPC], F32)   # [E-chunk part, chunk, n]
        nc.sync.dma_start(out=embT,
                          in_=embT_d.rearrange("(c p) n -> p c n", p=128))
        sg = singles.tile([128, 4, BPC], F32)
        nc.scalar.activation(sg, embT, AF.Sigmoid)
        semb = singles.tile([128, 4, BPC], F32)   # silu(emb)^T
        nc.vector.tensor_mul(semb, embT, sg)

        mT = singles.tile([128, 4, 640], F32)
        nc.sync.dma_start(out=mT, in_=mT_d.rearrange("(c p) m -> p c m", p=128))

        # bias_all[c, blk, n]: blk 0 -> b1; 1,2 -> b2 halves; 3,4 -> b3 halves
        bias_all = singles.tile([CIN, 5, BPC], F32)
        for blk in range(5):
            pb = psmall.tile([CIN, BPC], F32, tag="pp", name="pb")
            for e in range(4):
                nc.tensor.matmul(
                    pb, lhsT=mT[:, e, blk * 128:(blk + 1) * 128],
                    rhs=semb[:, e, :], start=(e == 0), stop=(e == 3),
                )
            nc.vector.tensor_copy(out=bias_all[:, blk, :], in_=pb)
        b1 = singles.tile([CIN, BPC], F32)
        nc.vector.tensor_scalar(b1, bias_all[:, 0, :], consts[:, 0:1], None, ALU.add)
        b2 = singles.tile([CIN, SC, BPC], F32)
        b3 = singles.tile([CIN, SC, BPC], F32)
        for i in range(SC):
            nc.vector.tensor_scalar(
                b2[:, i, :], bias_all[:, 1 + i, :], consts[:, 1 + i:2 + i], None, ALU.add)
            nc.vector.tensor_scalar(
                b3[:, i, :], bias_all[:, 3 + i, :], consts[:, 3 + i:4 + i], None, ALU.add)

        # ---------------- conv weights: load, sign, |w| means ----------------
        wraw = singles.tile([CIN, SC, 3, 3, CIN], F32)   # [ci, i, kh, kw, co]
        nc.sync.dma_start(
            out=wraw,
            in_=wT_d[:, :, :, :, :].rearrange("i kh kw ci co -> ci i kh kw co"))
        # DoubleRow weights: rows kh=0,2 paired per kw; kh=1 taps solo
        wdr = singles.tile([CIN, SC, 3, 2, CIN], F8)
        wsolo = singles.tile([CIN, SC, 3, CIN], F8)
        for i in range(SC):
            for kw in range(3):
                nc.scalar.activation(wdr[:, i, kw, 0, :], wraw[:, i, 0, kw, :], AF.Sign)
                nc.scalar.activation(wdr[:, i, kw, 1, :], wraw[:, i, 2, kw, :], AF.Sign)
                nc.scalar.activation(wsolo[:, i, kw, :], wraw[:, i, 1, kw, :], AF.Sign)
        wabs = singles.tile([CIN, SC, 3, 3, CIN], BF16)  # |w|
        nc.scalar.activation(wabs, wraw, AF.Abs)
        ones_vec = singles.tile([CIN, 1], BF16)
        nc.vector.memset(ones_vec, 1.0)
        sf = singles.tile([CIN, SC], F32)                # per-co mean|w|
        for i in range(SC):
            sfp = psmall.tile([CIN, BPC], F32, tag="pp", name="sfp")[:, 0:1]
            for t in range(9):
                kh, kw = t // 3, t % 3
                nc.tensor.matmul(
                    sfp, lhsT=wabs[:, i, kh, kw, :], rhs=ones_vec,
                    start=(t == 0), stop=(t == 8),
                )
            nc.scalar.activation(sf[:, i:i + 1], sfp, AF.Copy, scale=1.0 / 1152.0)

        # ---------------- activation planes + pooling ----------------
        # parity planes of act = sign(x + b1); odd planes zero-padded at idx 0
        pl_ee = singles.tile([CIN, BPC, 32, 32], F8)
        pl_eo = singles.tile([CIN, BPC, 32, 33], F8)
        pl_oe = singles.tile([CIN, BPC, 33, 32], F8)
        pl_oo = singles.tile([CIN, BPC, 33, 33], F8)
        for n in range(BPC):
            nc.gpsimd.memset(pl_eo[:, n, :, 0:1], 0.0)
            nc.gpsimd.memset(pl_oe[:, n, 0:1, :], 0.0)
            nc.gpsimd.memset(pl_oo[:, n, 0:1, :], 0.0)
            nc.gpsimd.memset(pl_oo[:, n, 1:33, 0:1], 0.0)

        xpsum = singles.tile([CIN, BPC, HO, WO], F32)    # 4 * avgpool2(x)

        # tap (kh, kw) -> plane and index offsets (see module docstring)
        def tap_ap(kh, kw, n, r0):
            rodd = kh != 1
            codd = kw != 1
            dr = 1 if kh == 2 else 0
            dc = 1 if kw == 2 else 0
            pl = {(False, False): pl_ee, (False, True): pl_eo,
                  (True, False): pl_oe, (True, True): pl_oo}[(rodd, codd)]
            nrow = TILE_N // WO  # 16
            return pl[:, n, r0 + dr:r0 + dr + nrow, dc:dc + WO]

        R = [singles.tile([CIN, BPC, HO * WO], F16, tag=f"R{i}", name=f"R{i}") for i in range(SC)]
        Sp = [singles.tile([CIN, BPC * TPI], F32, tag=f"Sp{i}", name=f"Sp{i}") for i in range(SC)]
        Qp = [singles.tile([CIN, BPC * TPI], F32, tag=f"Qp{i}", name=f"Qp{i}") for i in range(SC)]
        gstat4 = singles.tile([CIN, 4], F32)      # [S0,Q0,S1,Q1] global
        loc4 = singles.tile([CIN, 4], F32)
        cc_in4 = drampool.tile([CIN, 4], F32, tag="cci", name="cci")
        cc_out4 = drampool.tile([CIN, 4], F32, tag="cco", name="cco")

        NR = TILE_N // WO  # 16 output rows per PSUM tile

        def conv_tile(i, n, hb):
            ps = pspool.tile([CIN, TILE_N], F32, tag="ps", name="ps")
            r0 = hb * NR
            for kw in range(3):
                # rows kh=0 (idx r0) and kh=2 (idx r0+1) as a DoubleRow pair
                codd = kw != 1
                dc = 1 if kw == 2 else 0
                pl = pl_oo if codd else pl_oe
                base = pl[:, n, r0:r0 + NR, dc:dc + WO]
                rs = pl.shape[3]  # plane row stride in elements
                pair = bass.AP(
                    tensor=base.tensor, offset=base.offset,
                    ap=[list(base.ap[0]), [rs, 2]] + [list(x) for x in base.ap[1:]])
                nc.tensor.matmul(
                    ps, lhsT=wdr[:, i, kw, :, :], rhs=pair,
                    perf_mode=mybir.MatmulPerfMode.DoubleRow,
                    start=(kw == 0), stop=False,
                )
            for kw in range(3):
                nc.tensor.matmul(
                    ps, lhsT=wsolo[:, i, kw, :],
                    rhs=tap_ap(1, kw, n, r0),
                    start=False, stop=(kw == 2),
                )
            ti = n * TPI + hb
            rslice = R[i][:, n, hb * TILE_N:(hb + 1) * TILE_N]
            nc.scalar.activation(rslice, ps, AF.Identity,
                                 accum_out=Sp[i][:, ti:ti + 1])
            scr = ttrp.tile([CIN, TILE_N], F32, tag="ttr", name="scr")
            nc.vector.tensor_mul(scr, rslice, rslice)
            nc.vector.reduce_sum(out=Qp[i][:, ti:ti + 1], in_=scr, axis=AX.X)

        # All x DMAs issued upfront (4-deep pool); per image: plane signs
        # + pooling first, then the previous image's conv block - keeps
        # sign ops ahead of PSUM-evacs in the ACT FIFO so PE never waits.
        x_ts = []
        for n in range(BPC):
            x_t = xpool.tile([CIN, H, W], F32, tag="x_t", name=f"x_t{n}")
            nc.sync.dma_start(out=x_t, in_=x_d[n, :, :, :])
            x_ts.append(x_t)

        def plane_block(n):
            x_t = x_ts[n]
            xr = x_t.rearrange("p (h a) (w b) -> p h a w b", a=2, b=2)
            bn_ap = b1[:, n:n + 1]
            nc.scalar.activation(pl_ee[:, n, :, :], xr[:, :, 0, :, 0],
                                 AF.Sign, bias=bn_ap)
            nc.scalar.activation(pl_eo[:, n, :, 1:33], xr[:, :, 0, :, 1],
                                 AF.Sign, bias=bn_ap)
            nc.scalar.activation(pl_oe[:, n, 1:33, :], xr[:, :, 1, :, 0],
                                 AF.Sign, bias=bn_ap)
            nc.scalar.activation(pl_oo[:, n, 1:33, 1:33], xr[:, :, 1, :, 1],
                                 AF.Sign, bias=bn_ap)
            xw = x_t.rearrange("p h (w b) -> p h w b", b=2)
            t1 = t1pool.tile([CIN, H, WO], F32, tag="t1", name="t1")
            nc.vector.tensor_add(t1, xw[:, :, :, 0], xw[:, :, :, 1])
            t1r = t1.rearrange("p (h a) w -> p h a w", a=2)
            nc.vector.tensor_add(xpsum[:, n, :, :], t1r[:, :, 0, :], t1r[:, :, 1, :])

        def conv_block(n):
            for i in range(SC):
                for hb in range(TPI):
                    conv_tile(i, n, hb)

        plane_block(0)
        for n in range(1, BPC):
            plane_block(n)
            conv_block(n - 1)
        conv_block(BPC - 1)

        for i in range(SC):
            nc.vector.reduce_sum(out=loc4[:, 2 * i:2 * i + 1], in_=Sp[i], axis=AX.X)
            nc.vector.reduce_sum(out=loc4[:, 2 * i + 1:2 * i + 2], in_=Qp[i], axis=AX.X)

        nc.gpsimd.dma_start(out=cc_in4[:, :], in_=loc4)
        if USE_CC:
            nc.gpsimd.collective_compute(
                "AllReduce", ALU.add,
                replica_groups=[list(range(NC_CORES))],
                ins=[cc_in4[:, :].opt()], outs=[cc_out4[:, :].opt()],
            )
            nc.gpsimd.dma_start(out=gstat4, in_=cc_out4[:, :])
        else:
            nc.gpsimd.dma_start(out=gstat4, in_=cc_in4[:, :])

        if stage < 3:
            # dump R halves to out and stop
            for i in range(SC):
                for n in range(BPC):
                    dump = normp.tile([CIN, CH], F32, tag="xpD")
                    nc.vector.tensor_copy(out=dump, in_=R[i][:, n, :])
                    nc.sync.dma_start(
                        out=out_d[n, i * CIN:(i + 1) * CIN, :, :].rearrange(
                            "c h w -> c (h w)"),
                        in_=dump)
            return
        # ---------------- finalize per-channel A, D ----------------
        A = [singles.tile([CIN, 1], F32, tag=f"A{i}", name=f"A{i}") for i in range(SC)]
        Dn = [singles.tile([CIN, BPC], F32, tag=f"Dn{i}", name=f"Dn{i}") for i in range(SC)]
        for i in range(SC):
            tmp = singles.tile([CIN, 8], F32, tag=f"tmp{i}")
            mean = tmp[:, 6:7]
            e2 = tmp[:, 7:8]
            nc.scalar.activation(mean, gstat4[:, 2 * i:2 * i + 1], AF.Copy, scale=1.0 / NTOT)
            nc.scalar.activation(e2, gstat4[:, 2 * i + 1:2 * i + 2], AF.Copy, scale=1.0 / NTOT)
            # var = e2 - mean^2
            nc.vector.tensor_mul(tmp[:, 0:1], mean, mean)
            nc.vector.tensor_sub(tmp[:, 0:1], e2, tmp[:, 0:1])
            # z = var * sf^2 + eps
            nc.vector.tensor_mul(tmp[:, 1:2], sf[:, i:i + 1], sf[:, i:i + 1])
            nc.vector.tensor_mul(tmp[:, 1:2], tmp[:, 0:1], tmp[:, 1:2])
            nc.vector.tensor_scalar(tmp[:, 1:2], tmp[:, 1:2], BN_EPS, None, ALU.add)
            # r = 1/sqrt(z) via sqrt+reciprocal, then one Newton step:
            # r <- r * (1.5 - 0.5 * z * r^2)
            nc.scalar.activation(tmp[:, 2:3], tmp[:, 1:2], AF.Sqrt)
            nc.vector.reciprocal(tmp[:, 2:3], tmp[:, 2:3])
            nc.vector.tensor_mul(tmp[:, 3:4], tmp[:, 2:3], tmp[:, 2:3])
            nc.vector.tensor_mul(tmp[:, 3:4], tmp[:, 3:4], tmp[:, 1:2])
            nc.vector.tensor_scalar(tmp[:, 3:4], tmp[:, 3:4], -0.5, 1.5,
                                    ALU.mult, ALU.add)
            nc.vector.tensor_mul(tmp[:, 2:3], tmp[:, 2:3], tmp[:, 3:4])
            # A = gamma * sf * rstd
            nc.vector.tensor_mul(tmp[:, 4:5], consts[:, 5 + i:6 + i], sf[:, i:i + 1])
            nc.vector.tensor_mul(A[i], tmp[:, 4:5], tmp[:, 2:3])
            # Dn = bn_b - A*mean + b2      (per n)
            nc.vector.tensor_mul(tmp[:, 5:6], A[i], mean)
            nc.vector.scalar_tensor_tensor(
                out=tmp[:, 5:6], in0=tmp[:, 5:6], scalar=-1.0,
                in1=consts[:, 7 + i:8 + i], op0=ALU.mult, op1=ALU.add)
            nc.vector.tensor_scalar(Dn[i], b2[:, i, :], tmp[:, 5:6], None, ALU.add)

        # ---------------- normalize + prelu + bias3 + store ----------------
        for i in range(SC):
            a_ap = consts[:, 9 + i:10 + i]
            for n in range(BPC):
                k = i * BPC + n
                # fp16 intermediates: exact R, ~1e-3 abs noise on the
                # residual path - far below the binary-activation noise
                # floor; buys DVE 2x mode on the two STT passes.
                xpD = normp.tile([CIN, CH], F16, tag="xpD")
                xps = xpsum[:, n, :, :].rearrange("c h w -> c (h w)")
                nc.scalar.activation(xpD, xps, AF.Identity,
                                     bias=Dn[i][:, n:n + 1], scale=0.25)
                v = normp.tile([CIN, CH], F16, tag="v")
                nc.vector.scalar_tensor_tensor(
                    out=v, in0=R[i][:, n, :], scalar=A[i], in1=xpD,
                    op0=ALU.mult, op1=ALU.add)
                u = normp.tile([CIN, CH], F16, tag="u")
                nc.vector.scalar_tensor_tensor(
                    out=u, in0=v, scalar=a_ap, in1=v, op0=ALU.mult, op1=ALU.max)
                o = normp.tile([CIN, CH], F32, tag="o")
                if k % 2 == 0:
                    nc.vector.tensor_scalar(o, u, b3[:, i, n:n + 1], None, ALU.add)
                else:
                    nc.scalar.activation(o, u, AF.Identity, bias=b3[:, i, n:n + 1])
                nc.sync.dma_start(
                    out=out_d[n, i * CIN:(i + 1) * CIN, :, :].rearrange(
                        "c h w -> c (h w)"),
                    in_=o)


def build_nc(stage=3):
    nc = bacc.Bacc("TRN2", target_bir_lowering=False, debug=False)
    with tile.TileContext(nc) as tc:
        build_kernel_body(tc, stage)
    nc.finalize()
    return nc


def _prep_in_maps(x, emb, m1_w, m1_b, conv_w, conv_b, bn_g, bn_b,
                  m2_w, m2_b, prelu_a, m3_w, m3_b):
    """Host-side sharding + pure relayout (no arithmetic on tensor data)."""
    x = np.asarray(x, np.float32)
    conv_wT = np.ascontiguousarray(
        np.transpose(np.asarray(conv_w, np.float32), (0, 3, 4, 2, 1)))
    mT = np.ascontiguousarray(np.concatenate(
        [np.asarray(m1_w), np.asarray(m2_w), np.asarray(m3_w)], axis=0,
        dtype=np.float32).T)
    consts = np.zeros((CIN, 11), np.float32)
    consts[:, 0] = m1_b
    consts[:, 1] = m2_b[:CIN]
    consts[:, 2] = m2_b[CIN:]
    consts[:, 3] = m3_b[:CIN]
    consts[:, 4] = m3_b[CIN:]
    consts[:, 5] = bn_g[0]
    consts[:, 6] = bn_g[1]
    consts[:, 7] = bn_b[0]
    consts[:, 8] = bn_b[1]
    consts[:, 9] = prelu_a[:CIN]
    consts[:, 10] = prelu_a[CIN:]
    embT = np.ascontiguousarray(np.asarray(emb, np.float32).T)   # [E, B]
    in_maps = []
    for c in range(NC_CORES):
        in_maps.append({
            "x": np.ascontiguousarray(x[c * BPC:(c + 1) * BPC]),
            "embT": np.ascontiguousarray(embT[:, c * BPC:(c + 1) * BPC]),
            "conv_wT": conv_wT,
            "mT": mT,
            "consts": consts,
        })
    return in_maps


_CACHE = {}


def kernel(**inputs):
    from concourse.bass_utils import run_bass_kernel_spmd
    if "nc" not in _CACHE:
        _CACHE["nc"] = build_nc()
    nc = _CACHE["nc"]
    in_maps = _prep_in_maps(**{k: np.asarray(v) for k, v in inputs.items()})
    res = run_bass_kernel_spmd(nc, in_maps, list(range(NC_CORES)))
    return np.concatenate([res.results[c]["out"] for c in range(NC_CORES)], axis=0)


if __name__ == "__main__":
    build_nc()
    print("kernel build ok")
